# revision 14
# baseline (speedup 1.0000x reference)
"""db2 DWT LL band (separable, symmetric pad, stride 2) on Trainium2.

Input  x: (4, 64, 512, 512) f32  ->  Output: (4, 64, 257, 257) f32.

Approach: the 1D low-pass+downsample along an axis is y = x @ M with a banded
matrix M [512, 257] (4 nonzeros per interior column; symmetric-pad edge columns
use combined coefficients). The separable 2D LL band is out = M^T @ img @ M.

On the PE (out = lhsT.T @ rhs, lhsT transposed for free):
  stage A: zT = matmul(lhsT=img, rhs=M)   -> zT[w, h'] (no transpose needed)
  stage B: out = matmul(lhsT=zT,  rhs=M)  -> out[h', w'] (final layout!)

Band sparsity: a 128-row tile of M only covers ~63 output columns, so each
(row-tile, col-range) pair is one matmul with N~64 instead of 257 - a 4x cut
in PE column-cycles, which makes *exact fp32* matmuls (4 cyc/row) affordable.
Row tiles overlap by 2 rows (stride 126) so every output column's 4-tap window
lives in exactly one tile; output column ranges are disjoint, PSUM handles the
scatter via start-once/overwrite semantics.

The leftover output row h'=256 (h' has 257 rows, PE M-dim chunks are 128+128+1)
is computed for all images at once on the DVE from the gathered last two input
rows (symmetric boundary => z = b*x[510] + a*x[511], then the W-filter).

Sharding: pure data parallel - 256 (b,c) images, 32 per core on 8 cores.
"""

import numpy as np
from contextlib import ExitStack

import bass_rust
import concourse.bass as bass
import concourse.bacc as bacc
import concourse.tile as tile
from concourse import mybir
from concourse.bass_utils import run_bass_kernel_spmd

F32 = mybir.dt.float32
F32R = mybir.dt.float32r
F16 = mybir.dt.float16

# db2 dec_lo
H0 = -0.12940952255092145
H1 = 0.22414386804185735
H2 = 0.8365163037378079
H3 = 0.48296291314469025
CA = H1 + H2  # symmetric-edge combined coefficients
CB = H0 + H3

S = 512     # input height/width
O = 257     # output height/width per axis
N_CORES = 8
N_IMG = 32  # images per core (256 total / 8 cores)

# overlapping row tiles (stride 126) and the disjoint output-column range each covers
ROW_TILES = [(0, 128), (126, 254), (252, 380), (378, 506), (504, 512)]
COL_RANGES = [(0, 64), (64, 127), (127, 190), (190, 253), (253, 257)]

MULT = mybir.AluOpType.mult
ADD = mybir.AluOpType.add


def build_filter_matrix() -> np.ndarray:
    m = np.zeros((S, O), dtype=np.float32)
    m[0, 0], m[1, 0] = CA, CB
    for i in range(1, 256):
        m[2 * i - 2, i] = H3
        m[2 * i - 1, i] = H2
        m[2 * i, i] = H1
        m[2 * i + 1, i] = H0
    m[510, 256], m[511, 256] = CB, CA
    return m


def _emit_row256(nc, tc, ctx, x, y, n_img):
    """out[:, 256, :] for all images: z = CB*x[510] + CA*x[511], then W-filter."""
    rp = ctx.enter_context(tc.tile_pool(name="r256", bufs=1))
    r510 = rp.tile([n_img, S], F32, tag="r510")
    nc.sync.dma_start(r510[:], x[:, 510, :].bitcast(F32))
    r511 = rp.tile([n_img, S], F32, tag="r511")
    nc.sync.dma_start(r511[:], x[:, 511, :].bitcast(F32))
    t0 = rp.tile([n_img, S], F32, tag="t0")
    nc.vector.tensor_scalar_mul(t0[:], r510[:], CB)
    z = rp.tile([n_img, S], F32, tag="z")
    nc.vector.scalar_tensor_tensor(z[:], r511[:], CA, t0[:], op0=MULT, op1=ADD)

    a0 = rp.tile([n_img, 255], F32, tag="a0")
    a1 = rp.tile([n_img, 255], F32, tag="a1")
    y256 = rp.tile([n_img, O], F32, tag="y256")
    # interior columns 1..255: y[i] = H3*z[2i-2] + H2*z[2i-1] + H1*z[2i] + H0*z[2i+1]
    nc.vector.tensor_scalar_mul(a0[:], z[:, 0:510:2], H3)
    nc.vector.scalar_tensor_tensor(a1[:], z[:, 1:511:2], H2, a0[:], op0=MULT, op1=ADD)
    nc.vector.scalar_tensor_tensor(a0[:], z[:, 2:512:2], H1, a1[:], op0=MULT, op1=ADD)
    nc.vector.scalar_tensor_tensor(y256[:, 1:256], z[:, 3:512:2], H0, a0[:], op0=MULT, op1=ADD)
    # edge columns
    c0 = rp.tile([n_img, 1], F32, tag="c0")
    nc.vector.tensor_scalar_mul(c0[:], z[:, 0:1], CA)
    nc.vector.scalar_tensor_tensor(y256[:, 0:1], z[:, 1:2], CB, c0[:], op0=MULT, op1=ADD)
    c1 = rp.tile([n_img, 1], F32, tag="c1")
    nc.vector.tensor_scalar_mul(c1[:], z[:, 510:511], CB)
    nc.vector.scalar_tensor_tensor(y256[:, 256:257], z[:, 511:512], CA, c1[:], op0=MULT, op1=ADD)
    nc.sync.dma_start(y[:, 256, :], y256[:])


def build_nc_fp32_banded(n_img: int = N_IMG):
    nc = bacc.Bacc("TRN2", target_bir_lowering=False, debug=False, num_devices=N_CORES)
    x = nc.declare_dram_parameter("x", [n_img, S, S], F32, isOutput=False)
    m = nc.declare_dram_parameter("m", [S, O], F32, isOutput=False)
    y = nc.declare_dram_parameter("y", [n_img, O, O], F32, isOutput=True)

    with tile.TileContext(nc) as tc:
        with ExitStack() as ctx:
            mp = ctx.enter_context(tc.tile_pool(name="mp", bufs=1))
            xp = ctx.enter_context(tc.tile_pool(name="xp", bufs=3))
            zp = ctx.enter_context(tc.tile_pool(name="zp", bufs=2))
            op = ctx.enter_context(tc.tile_pool(name="op", bufs=2))
            psa = ctx.enter_context(tc.tile_pool(name="psa", bufs=1, space="PSUM"))
            psb = ctx.enter_context(tc.tile_pool(name="psb", bufs=1, space="PSUM"))

            # banded filter blocks: mb[j] = M[rows RT[j], cols CR[j]]
            mb = []
            for j, (r0, r1) in enumerate(ROW_TILES):
                c0, c1 = COL_RANGES[j]
                t = mp.tile([r1 - r0, c1 - c0], F32, tag=f"m{j}")
                nc.sync.dma_start(t[:], m[r0:r1, c0:c1])
                mb.append(t)

            prev = None  # (zt tiles, image idx) pending stage B
            for n in range(n_img + 1):
                pz = None
                if n < n_img:
                    xt = []
                    for j, (r0, r1) in enumerate(ROW_TILES):
                        t = xp.tile([r1 - r0, S], F32, tag=f"x{j}")
                        nc.sync.dma_start(t[:], x[n, r0:r1, :])
                        xt.append(t)
                    # stage A: zT tile per w-slice; disjoint col writes per row-tile
                    pz = []
                    for i, (w0, w1) in enumerate(ROW_TILES):
                        p = psa.tile([w1 - w0, 256], F32, tag=f"z{i}")
                        for j, (r0, r1) in enumerate(ROW_TILES):
                            c0, c1 = COL_RANGES[j]
                            c1a = min(c1, 256)
                            nc.tensor.matmul(
                                p[:, c0:c1a], xt[j][:, w0:w1], mb[j][:, 0:c1a - c0],
                                start=(j == 0), stop=(j == len(ROW_TILES) - 1))
                        pz.append(p)
                po = None
                if prev is not None:
                    ztp, _ = prev
                    po = []
                    for mbi in range(2):
                        p = psb.tile([128, O], F32, tag=f"o{mbi}")
                        for i, (w0, w1) in enumerate(ROW_TILES):
                            c0, c1 = COL_RANGES[i]
                            nc.tensor.matmul(
                                p[:, c0:c1], ztp[i][:, mbi * 128:(mbi + 1) * 128], mb[i][:],
                                start=(i == 0), stop=(i == len(ROW_TILES) - 1))
                        po.append(p)
                if n < n_img:
                    zt = []
                    for i, (w0, w1) in enumerate(ROW_TILES):
                        t = zp.tile([w1 - w0, 256], F32, tag=f"zt{i}")
                        eng = nc.vector.tensor_copy if i % 2 == 0 else nc.scalar.copy
                        eng(t[:], pz[i][:])
                        zt.append(t)
                if prev is not None:
                    _, pn = prev
                    for mbi in range(2):
                        t = op.tile([128, O], F32, tag=f"ot{mbi}")
                        eng = nc.scalar.copy if mbi == 0 else nc.vector.tensor_copy
                        eng(t[:], po[mbi][:])
                        nc.sync.dma_start(y[pn, mbi * 128:(mbi + 1) * 128, :], t[:])
                prev = (zt, n) if n < n_img else None

            _emit_row256(nc, tc, ctx, x, y, n_img)
    nc.compile()
    return nc


def build_nc_fp32r_dense(n_img: int = N_IMG):
    nc = bacc.Bacc("TRN2", target_bir_lowering=False, debug=False, num_devices=N_CORES)
    x = nc.declare_dram_parameter("x", [n_img, S, S], F32R, isOutput=False)
    m = nc.declare_dram_parameter("m", [S, O], F32, isOutput=False)
    y = nc.declare_dram_parameter("y", [n_img, O, O], F32, isOutput=True)

    with tile.TileContext(nc) as tc:
        with ExitStack() as ctx:
            mp = ctx.enter_context(tc.tile_pool(name="mp", bufs=1))
            xp = ctx.enter_context(tc.tile_pool(name="xp", bufs=3))
            zp = ctx.enter_context(tc.tile_pool(name="zp", bufs=2))
            op = ctx.enter_context(tc.tile_pool(name="op", bufs=2))
            psa = ctx.enter_context(tc.tile_pool(name="psa", bufs=1, space="PSUM"))
            psb = ctx.enter_context(tc.tile_pool(name="psb", bufs=1, space="PSUM"))

            # M chunks as fp32r (round once via DVE). 258 cols: fp32r moving
            # free-dim must be even; col 257 is zero padding.
            mr = []
            for j in range(4):
                tf = mp.tile([128, O], F32, tag=f"mf{j}")
                nc.sync.dma_start(tf[:], m[j * 128:(j + 1) * 128, :])
                tr = mp.tile([128, 258], F32, tag=f"mr{j}")
                nc.vector.memset(tr[:, 256:258], 0.0)
                nc.vector.tensor_copy(tr[:, 0:257].bitcast(F32R), tf[:])
                mr.append(tr)

            prev = None
            for n in range(n_img + 1):
                pz = None
                if n < n_img:
                    xr = []
                    for j in range(4):
                        t = xp.tile([128, S], F32R, tag=f"x{j}")
                        nc.sync.dma_start(t[:], x[n, j * 128:(j + 1) * 128, :])
                        xr.append(t)
                    pz = []
                    for i in range(4):
                        p = psa.tile([128, 256], F32, tag=f"z{i}")
                        for j in range(4):
                            nc.tensor.matmul(
                                p[:], xr[j][:, i * 128:(i + 1) * 128],
                                mr[j][:, 0:256].bitcast(F32R),
                                start=(j == 0), stop=(j == 3))
                        pz.append(p)
                po = None
                if prev is not None:
                    ztp, _ = prev
                    po = []
                    for mbi in range(2):
                        p = psb.tile([128, 258], F32, tag=f"o{mbi}")
                        for i in range(4):
                            nc.tensor.matmul(
                                p[:], ztp[i][:, mbi * 128:(mbi + 1) * 128].bitcast(F32R),
                                mr[i][:].bitcast(F32R),
                                start=(i == 0), stop=(i == 3))
                        po.append(p)
                if n < n_img:
                    zt = []
                    for i in range(4):
                        t = zp.tile([128, 256], F32, tag=f"zt{i}")
                        eng = nc.vector.tensor_copy if i % 2 == 0 else nc.scalar.copy
                        eng(t[:].bitcast(F32R), pz[i][:])
                        zt.append(t)
                if prev is not None:
                    _, pn = prev
                    for mbi in range(2):
                        t = op.tile([128, O], F32, tag=f"ot{mbi}")
                        eng = nc.scalar.copy if mbi == 0 else nc.vector.tensor_copy
                        eng(t[:], po[mbi][:, 0:257])
                        nc.sync.dma_start(y[pn, mbi * 128:(mbi + 1) * 128, :], t[:])
                prev = (zt, n) if n < n_img else None

            _emit_row256(nc, tc, ctx, x, y, n_img)
    nc.compile()
    return nc



def build_nc_fp16_q(n_img: int = N_IMG, po_bufs: int = 1, psa4e_bufs: int = 2,
                    po_single: bool = False, drain_lag: int = 0):
    """fp16 end-to-end, banded stride-126 tiling, quad-grouped output DMAs.

    Input x is pre-rounded to fp16 on CPU (free); output y is fp16 in HBM and
    upcast on CPU. All matmuls fp16 (1 cyc/row on PE) with f32 PSUM accum.

    Layouts (per quad of Q=4 images):
      xtS [128, Q*4*512]: xtS[p, (k*4+b)*512+w] = x[n0+k, 126b+p, w]
          (overlapping stride-126 h-blocks b=0..3, raw-AP DMA per image)
      xt4 [8, Q*512]:     xt4[p, k*512+w]       = x[n0+k, 504+p, w]
      mt5 [128, 5*257]:   mt5[p, b*257+c]       = M[126b+p, c]  (b=4: p<8)

    PSUM (8 banks): psA0 x2 = zT w-slices 0,1 | psA1 x2 = slices 2,3 |
      psA4E x2 = slice 4 ([0:8, 0:256]) + zT h'=256 edge col per slice
      ([:, 256+s]) | po = out h' rows, both 128-halves | perE = out row 256
      (per quad, [0:Q, 0:257]) + out w'=256 edge col per image ([:, 257+2k+hb]).
    All matmuls keep base partition 0 (HW requires base in {0,32,64}); psum
    group start/stop land on full-128-partition matmuls so the zero-region
    flags open and clear consistently.
    """
    assert n_img % 4 == 0
    Q = 4
    nc = bacc.Bacc("TRN2", target_bir_lowering=False, debug=False, num_devices=N_CORES)
    x = nc.declare_dram_parameter("x", [n_img, S, S], F16, isOutput=False)
    m = nc.declare_dram_parameter("m", [S, O], F16, isOutput=False)
    y = nc.declare_dram_parameter("y", [n_img, O, O], F16, isOutput=True)

    def emit_group(mms):
        for q, (o_, l_, r_) in enumerate(mms):
            nc.tensor.matmul(o_, l_, r_, start=(q == 0), stop=(q == len(mms) - 1))

    with tile.TileContext(nc) as tc:
        with ExitStack() as ctx:
            mp = ctx.enter_context(tc.tile_pool(name="mp", bufs=1))
            xp = ctx.enter_context(tc.tile_pool(name="xp", bufs=3))
            zp = ctx.enter_context(tc.tile_pool(name="zp", bufs=2))
            op = ctx.enter_context(tc.tile_pool(name="op", bufs=2))
            ps = ctx.enter_context(tc.tile_pool(name="ps", bufs=1, space="PSUM"))

            def K_(b):
                return ROW_TILES[b][1] - ROW_TILES[b][0]

            mt5 = mp.tile([128, 5 * O], F16, tag="mt5")
            for b in range(5):
                r0, r1 = ROW_TILES[b]
                nc.sync.dma_start(mt5[0:r1 - r0, b * O:(b + 1) * O], m[r0:r1, :])

            def mmain(b, cap=None):
                c0, c1 = COL_RANGES[b]
                if cap is not None:
                    c1 = min(c1, cap)
                return mt5[0:K_(b), b * O + c0: b * O + c1]

            medge = mt5[0:8, 4 * O + 256: 4 * O + 257]

            n_quads = n_img // Q
            quads = {}    # g -> dict(xtS, xt4, zte, opt, perE, poE_last)
            pending = {}  # image t -> dict(ztA, ztB, zt4, g, k)
            pending_drain = {}  # image t -> (po, opt, k): deferred po drain

            def issue_quad_dmas(g):
                n0 = g * Q
                xtS = xp.tile([128, Q * 4 * S], F16, tag="xtS", name=f"xtS{g}")
                for k in range(Q):
                    src = x[n0 + k].copy()
                    src.ap = bass_rust.VecI64Pair(
                        [[S, 128], [126 * S, 4], [1, S]])
                    nc.sync.dma_start(
                        xtS[:, k * 4 * S:(k + 1) * 4 * S].rearrange(
                            "p (b w) -> p b w", b=4), src)
                xt4 = xp.tile([8, Q * S], F16, tag="xt4", name=f"xt4{g}")
                nc.sync.dma_start(
                    xt4[:].rearrange("p (n w) -> p n w", n=Q),
                    x[n0:n0 + Q, 504:512, :].rearrange("n p w -> p n w"))
                quads[g] = {"xtS": xtS, "xt4": xt4}

            def stage_a(t):
                g, k = t // Q, t % Q
                qd = quads[g]
                xtS, xt4 = qd["xtS"], qd["xt4"]
                if k == 0:
                    qd["zte"] = zp.tile([128, Q * 5], F16, tag="zte",
                                        name=f"zte{g}")
                zte = qd["zte"]

                def xb(b, w0, w1):
                    if b == 4:
                        return xt4[0:8, k * S + w0: k * S + w1]
                    o0 = (k * 4 + b) * S
                    return xtS[:, o0 + w0: o0 + w1]

                pzA = ps.tile([128, 512], F32, tag="psA0", bufs=2)
                pzB = ps.tile([128, 512], F32, tag="psA1", bufs=2)
                pz4E = ps.tile([128, 512], F32, tag="psA4E", bufs=psa4e_bufs)
                gA, gB, g4 = [], [], []
                edges = []
                for s in range(5):
                    w0, w1 = ROW_TILES[s]
                    P = w1 - w0
                    dst, cb, grp = [(pzA, 0, gA), (pzA, 256, gA),
                                    (pzB, 0, gB), (pzB, 256, gB),
                                    (pz4E, 0, g4)][s]
                    for b in range(5):
                        c0, c1 = COL_RANGES[b]
                        c1a = min(c1, 256)
                        grp.append((dst[0:P, cb + c0: cb + c1a],
                                    xb(b, w0, w1), mmain(b, cap=256)))
                    edges.append((pz4E[0:P, 256 + s: 257 + s],
                                  xb(4, w0, w1), medge))
                emit_group(gA)
                emit_group(gB)
                # psA4E bank group: P=128 edge first (start) and last (stop);
                # the P=8 slice-4 work sits mid-group
                emit_group([edges[0]] + g4 + [edges[4], edges[1],
                                              edges[2], edges[3]])

                # zT copies stay off Pool so the stage-B output copies
                # (Pool) can't back-pressure the A->B chain via in-order
                # engine streams
                ztA = zp.tile([128, 512], F16, tag="ztA", bufs=3)
                nc.vector.tensor_copy(ztA[:], pzA[:])
                ztB = zp.tile([128, 512], F16, tag="ztB", bufs=3)
                nc.scalar.copy(ztB[:], pzB[:])
                zt4 = zp.tile([8, 256], F16, tag="zt4", bufs=3)
                nc.vector.tensor_copy(zt4[:], pz4E[0:8, 0:256])
                nc.scalar.copy(zte[:, k * 5:k * 5 + 4], pz4E[:, 256:260])
                nc.scalar.copy(zte[0:8, k * 5 + 4:k * 5 + 5],
                               pz4E[0:8, 260:261])
                pending[t] = {"ztA": ztA, "ztB": ztB, "zt4": zt4}

            def stage_b(t):
                g, k = t // Q, t % Q
                qd = quads[g]
                pd = pending.pop(t)
                ztA, ztB, zt4 = pd["ztA"], pd["ztB"], pd["zt4"]
                if k == 0:
                    qd["opt"] = op.tile([128, Q * 2 * O], F16, tag="opt",
                                        name=f"opt{g}", bufs=3)
                    qd["perE"] = ps.tile([128, 512], F32, tag="perE",
                                         name=f"perE{g}")
                opt, perE = qd["opt"], qd["perE"]

                def zb(s, h0, h1):
                    if s < 2:
                        return ztA[:, s * 256 + h0: s * 256 + h1]
                    if s < 4:
                        return ztB[:, (s - 2) * 256 + h0: (s - 2) * 256 + h1]
                    return zt4[0:8, h0:h1]

                po = ps.tile([128, 512], F32, tag="po", bufs=po_bufs)
                gb = []
                for hb in range(2):
                    for s in range(5):
                        c0, c1 = COL_RANGES[s]
                        c1a = min(c1, 256)
                        gb.append((po[:, hb * 256 + c0: hb * 256 + c1a],
                                   zb(s, hb * 128, hb * 128 + 128),
                                   mmain(s, cap=256)))
                emit_group(gb)
                # out w'=256 edge col -> perE[:, 257+2k+hb]; the perE bank
                # group opens at image 0 and closes after the per-quad
                # edge-row matmuls (deferred last poE carries the stop)
                for hb in range(2):
                    mm = (perE[:, 257 + 2 * k + hb: 258 + 2 * k + hb],
                          zb(4, hb * 128, hb * 128 + 128), medge)
                    if k == Q - 1 and hb == 1:
                        qd["poE_last"] = mm
                    else:
                        nc.tensor.matmul(*mm, start=(k == 0 and hb == 0),
                                         stop=False)

                # Pool/GPSIMD can't read PSUM -> DVE/Act do the po drain
                if drain_lag:
                    pending_drain[t] = (po, opt, k)
                elif po_single:
                    # one instruction, alternate engine per image
                    eng = nc.vector.tensor_copy if t % 2 == 0 else nc.scalar.copy
                    eng(opt[:, k * 2 * O:(k + 1) * 2 * O].rearrange(
                            "p (b w) -> p b w", b=2)[:, :, 0:256],
                        po[:].rearrange("p (b w) -> p b w", b=2))
                else:
                    nc.vector.tensor_copy(
                        opt[:, k * 2 * O: k * 2 * O + 256], po[:, 0:256])
                    nc.scalar.copy(
                        opt[:, k * 2 * O + O: k * 2 * O + O + 256], po[:, 256:512])

            def quad_tail(g):
                qd = quads[g]
                zte, opt, perE = qd["zte"], qd["opt"], qd["perE"]
                # out row 256 (h'=256) for the quad, from zte
                for s in range(5):
                    P = K_(s)
                    c0, c1 = COL_RANGES[s]
                    nc.tensor.matmul(perE[0:Q, c0:c1], zte[0:P, s:Q * 5:5],
                                     mmain(s), start=False, stop=False)
                nc.tensor.matmul(*qd["poE_last"], start=False, stop=True)

                ert = op.tile([Q, O], F16, tag="ert", bufs=3)
                nc.scalar.copy(ert[:], perE[0:Q, 0:257])
                qd["ert"] = ert
                nc.scalar.copy(
                    opt[:].rearrange("p (n b w) -> p n b w", n=Q, b=2)[:, :, :, 256],
                    perE[:, 257:257 + 2 * Q].rearrange("p (n b) -> p n b", n=Q))

            def quad_out_dmas(g):
                # one step after quad_tail: by now every opt/ert write has
                # landed, so these never block SP's in-order queue (keeping
                # input prefetch flowing)
                n0 = g * Q
                qd = quads[g]
                opt, ert = qd["opt"], qd["ert"]
                nc.sync.dma_start(y[n0:n0 + Q, 256, :], ert[:])
                for hb in range(2):
                    nc.sync.dma_start(
                        y[n0:n0 + Q, hb * 128:(hb + 1) * 128, :].rearrange(
                            "n p w -> p n w"),
                        opt[:].rearrange("p (n b w) -> p n b w",
                                         n=Q, b=2)[:, :, hb, :])
                del quads[g]

            # software pipeline with lag 2: A(t) and A(t+1) run before
            # B(t), so the zT psum->sbuf copies of image t complete well
            # before stage B consumes them and the PE never stalls;
            # input DMAs prefetch one full quad ahead
            def drain_po(t):
                po, opt_, k_ = pending_drain.pop(t)
                eng = nc.vector.tensor_copy if t % 2 == 0 else nc.scalar.copy
                eng(opt_[:, k_ * 2 * O:(k_ + 1) * 2 * O].rearrange(
                        "p (b w) -> p b w", b=2)[:, :, 0:256],
                    po[:].rearrange("p (b w) -> p b w", b=2))

            issue_quad_dmas(0)
            if n_quads > 1:
                issue_quad_dmas(1)
            for t in range(n_img + 6):
                if drain_lag and (t - 2 - drain_lag) in pending_drain:
                    drain_po(t - 2 - drain_lag)
                if t < n_img:
                    if t % Q == 0 and t // Q + 2 < n_quads:
                        issue_quad_dmas(t // Q + 2)
                    stage_a(t)
                if 2 <= t < n_img + 2:
                    stage_b(t - 2)
                    if (t - 2) % Q == Q - 1:
                        quad_tail((t - 2) // Q)
                if t >= 4 and (t - 4) % Q == Q - 1:
                    quad_out_dmas((t - 4) // Q)
    nc.compile()
    return nc


def build_nc_fp16_v2(n_img: int = N_IMG):
    """Non-overlapping 128-row tiling variant of fp16_q.

    Both filter axes are tiled in 4 aligned 128-chunks (no stride-126
    overlap): band-boundary output columns (64/128/192) get contributions
    from two adjacent chunks, accumulated in PSUM via has_written adds.
    Kills the 8-row tail path entirely: the h'=256 / w'=256 edge columns
    (M rows 510/511) live in chunk 3, so the only extra matmuls are the
    per-slice zT-edge column (K=128, N=1) into a small pzE bank.

    Per image: 1 input DMA (fully contiguous 512KB), 20+10 matmuls,
    4 psum->sbuf copies. PSUM: pzA x2, pzB x2, pzE x1, po x2, perE x1 = 8.
    The po drain is deferred one pipeline step so it never convoys the
    stage-A copies on the in-order DVE/Act queues.
    """
    assert n_img % 4 == 0
    Q = 4
    HB = [(0, 128), (128, 256), (256, 384), (384, 512)]   # chunks (both axes)
    CR = [(0, 65), (64, 129), (128, 193), (192, 257)]     # cols per chunk
    nc = bacc.Bacc("TRN2", target_bir_lowering=False, debug=False, num_devices=N_CORES)
    x = nc.declare_dram_parameter("x", [n_img, S, S], F16, isOutput=False)
    m = nc.declare_dram_parameter("m", [S, O], F16, isOutput=False)
    y = nc.declare_dram_parameter("y", [n_img, O, O], F16, isOutput=True)

    def emit_group(mms):
        for q, (o_, l_, r_) in enumerate(mms):
            nc.tensor.matmul(o_, l_, r_, start=(q == 0), stop=(q == len(mms) - 1))

    with tile.TileContext(nc) as tc:
        with ExitStack() as ctx:
            mp = ctx.enter_context(tc.tile_pool(name="mp", bufs=1))
            xp = ctx.enter_context(tc.tile_pool(name="xp", bufs=3))
            zp = ctx.enter_context(tc.tile_pool(name="zp", bufs=2))
            op = ctx.enter_context(tc.tile_pool(name="op", bufs=2))
            ps = ctx.enter_context(tc.tile_pool(name="ps", bufs=1, space="PSUM"))

            mt4 = mp.tile([128, 4 * O], F16, tag="mt4")
            for b in range(4):
                nc.sync.dma_start(mt4[:, b * O:(b + 1) * O],
                                  m[b * 128:(b + 1) * 128, :])

            def mmain(b, cap=None):
                c0, c1 = CR[b]
                if cap is not None:
                    c1 = min(c1, cap)
                return mt4[:, b * O + c0: b * O + c1]

            medge = mt4[:, 3 * O + 256: 3 * O + 257]  # M[384:512, 256]

            n_quads = n_img // Q
            quads = {}
            pending = {}
            pending_drain = {}

            def issue_quad_dmas(g):
                n0 = g * Q
                xtS = xp.tile([128, Q * 4 * S], F16, tag="xtS", name=f"xtS{g}")
                for k in range(Q):
                    src = x[n0 + k].copy()
                    src.ap = bass_rust.VecI64Pair(
                        [[S, 128], [128 * S, 4], [1, S]])
                    nc.sync.dma_start(
                        xtS[:, k * 4 * S:(k + 1) * 4 * S].rearrange(
                            "p (b w) -> p b w", b=4), src)
                quads[g] = {"xtS": xtS}

            def stage_a(t):
                g, k = t // Q, t % Q
                qd = quads[g]
                xtS = qd["xtS"]
                if k == 0:
                    qd["zte"] = zp.tile([128, Q * 4], F16, tag="zte",
                                        name=f"zte{g}")
                zte = qd["zte"]

                def xb(b, w0, w1):
                    o0 = (k * 4 + b) * S
                    return xtS[:, o0 + w0: o0 + w1]

                pzA = ps.tile([128, 512], F32, tag="psA0", bufs=2)
                pzB = ps.tile([128, 512], F32, tag="psA1", bufs=2)
                pzE = ps.tile([128, 4], F32, tag="psE", bufs=1)
                gA, gB, gE = [], [], []
                for s in range(4):
                    w0, w1 = HB[s]
                    dst, cb, grp = [(pzA, 0, gA), (pzA, 256, gA),
                                    (pzB, 0, gB), (pzB, 256, gB)][s]
                    for b in range(4):
                        c0, c1 = CR[b]
                        c1a = min(c1, 256)
                        grp.append((dst[:, cb + c0: cb + c1a],
                                    xb(b, w0, w1), mmain(b, cap=256)))
                    gE.append((pzE[:, s:s + 1], xb(3, w0, w1), medge))
                emit_group(gA)
                emit_group(gB)
                emit_group(gE)

                ztA = zp.tile([128, 512], F16, tag="ztA", bufs=3)
                nc.vector.tensor_copy(ztA[:], pzA[:])
                ztB = zp.tile([128, 512], F16, tag="ztB", bufs=3)
                nc.scalar.copy(ztB[:], pzB[:])
                nc.scalar.copy(zte[:, k * 4:(k + 1) * 4], pzE[:])
                pending[t] = {"ztA": ztA, "ztB": ztB}

            def stage_b(t):
                g, k = t // Q, t % Q
                qd = quads[g]
                pd = pending.pop(t)
                ztA, ztB = pd["ztA"], pd["ztB"]
                if k == 0:
                    qd["opt"] = op.tile([128, Q * 2 * O], F16, tag="opt",
                                        name=f"opt{g}", bufs=3)
                    qd["perE"] = ps.tile([128, 512], F32, tag="perE",
                                         name=f"perE{g}")
                opt, perE = qd["opt"], qd["perE"]

                def zb(s, h0, h1):
                    if s < 2:
                        return ztA[:, s * 256 + h0: s * 256 + h1]
                    return ztB[:, (s - 2) * 256 + h0: (s - 2) * 256 + h1]

                po = ps.tile([128, 512], F32, tag="po", bufs=2)
                gb = []
                for hb in range(2):
                    for s in range(4):
                        c0, c1 = CR[s]
                        c1a = min(c1, 256)
                        gb.append((po[:, hb * 256 + c0: hb * 256 + c1a],
                                   zb(s, hb * 128, hb * 128 + 128),
                                   mmain(s, cap=256)))
                emit_group(gb)
                # out w'=256 edge col -> perE[:, 257+2k+hb]
                for hb in range(2):
                    mm = (perE[:, 257 + 2 * k + hb: 258 + 2 * k + hb],
                          zb(3, hb * 128, hb * 128 + 128), medge)
                    if k == Q - 1 and hb == 1:
                        qd["poE_last"] = mm
                    else:
                        nc.tensor.matmul(*mm, start=(k == 0 and hb == 0),
                                         stop=False)
                pending_drain[t] = (po, opt, k)

            def drain_po(t):
                po, opt_, k_ = pending_drain.pop(t)
                eng = nc.vector.tensor_copy if t % 2 == 0 else nc.scalar.copy
                eng(opt_[:, k_ * 2 * O:(k_ + 1) * 2 * O].rearrange(
                        "p (b w) -> p b w", b=2)[:, :, 0:256],
                    po[:].rearrange("p (b w) -> p b w", b=2))

            def quad_tail(g):
                qd = quads[g]
                zte, opt, perE = qd["zte"], qd["opt"], qd["perE"]
                # out row h'=256 for the quad (incl. the w'=256 corner)
                for s in range(4):
                    c0, c1 = CR[s]
                    nc.tensor.matmul(perE[0:Q, c0:c1], zte[:, s:Q * 4:4],
                                     mmain(s), start=False, stop=False)
                nc.tensor.matmul(*qd["poE_last"], start=False, stop=True)

                ert = op.tile([Q, O], F16, tag="ert", bufs=3)
                nc.scalar.copy(ert[:], perE[0:Q, 0:257])
                qd["ert"] = ert
                nc.scalar.copy(
                    opt[:].rearrange("p (n b w) -> p n b w", n=Q, b=2)[:, :, :, 256],
                    perE[:, 257:257 + 2 * Q].rearrange("p (n b) -> p n b", n=Q))

            def quad_out_dmas(g):
                n0 = g * Q
                qd = quads[g]
                opt, ert = qd["opt"], qd["ert"]
                nc.sync.dma_start(y[n0:n0 + Q, 256, :], ert[:])
                for hb in range(2):
                    nc.sync.dma_start(
                        y[n0:n0 + Q, hb * 128:(hb + 1) * 128, :].rearrange(
                            "n p w -> p n w"),
                        opt[:].rearrange("p (n b w) -> p n b w",
                                         n=Q, b=2)[:, :, hb, :])
                del quads[g]

            issue_quad_dmas(0)
            if n_quads > 1:
                issue_quad_dmas(1)
            for t in range(n_img + 7):
                if (t - 3) in pending_drain:
                    drain_po(t - 3)
                if t < n_img:
                    if t % Q == 0 and t // Q + 2 < n_quads:
                        issue_quad_dmas(t // Q + 2)
                    stage_a(t)
                if 2 <= t < n_img + 2:
                    stage_b(t - 2)
                    if (t - 2) % Q == Q - 1:
                        quad_tail((t - 2) // Q)
                if t >= 5 and (t - 5) % Q == Q - 1:
                    quad_out_dmas((t - 5) // Q)
    nc.compile()
    return nc


def build_nc_dma_only(n_img: int = N_IMG):
    """DMA floor probe: same input/output DMA traffic as fp16_q, no compute.

    Output data is garbage (a zero tile DMA'd out) - only for run_time.
    """
    assert n_img % 4 == 0
    Q = 4
    nc = bacc.Bacc("TRN2", target_bir_lowering=False, debug=False, num_devices=N_CORES)
    x = nc.declare_dram_parameter("x", [n_img, S, S], F16, isOutput=False)
    m = nc.declare_dram_parameter("m", [S, O], F16, isOutput=False)
    y = nc.declare_dram_parameter("y", [n_img, O, O], F16, isOutput=True)

    with tile.TileContext(nc) as tc:
        with ExitStack() as ctx:
            mp = ctx.enter_context(tc.tile_pool(name="mp", bufs=1))
            xp = ctx.enter_context(tc.tile_pool(name="xp", bufs=3))
            op = ctx.enter_context(tc.tile_pool(name="op", bufs=1))
            mt = mp.tile([128, O], F16, tag="mt")
            nc.sync.dma_start(mt[:], m[0:128, :])
            opt = op.tile([128, Q * 2 * O], F16, tag="opt")
            nc.vector.memset(opt[:], 0.0)
            ert = op.tile([Q, O], F16, tag="ert")
            nc.vector.memset(ert[:], 0.0)
            for g in range(n_img // Q):
                n0 = g * Q
                xtS = xp.tile([128, Q * 4 * S], F16, tag="xtS", name=f"xtS{g}")
                for k in range(Q):
                    src = x[n0 + k].copy()
                    src.ap = bass_rust.VecI64Pair(
                        [[S, 128], [128 * S, 4], [1, S]])
                    nc.sync.dma_start(
                        xtS[:, k * 4 * S:(k + 1) * 4 * S].rearrange(
                            "p (b w) -> p b w", b=4), src)
                nc.sync.dma_start(y[n0:n0 + Q, 256, :], ert[:])
                for hb in range(2):
                    nc.sync.dma_start(
                        y[n0:n0 + Q, hb * 128:(hb + 1) * 128, :].rearrange(
                            "n p w -> p n w"),
                        opt[:].rearrange("p (n b w) -> p n b w",
                                         n=Q, b=2)[:, :, hb, :])
    nc.compile()
    return nc


_BUILDERS = {
    "fp32_banded": build_nc_fp32_banded,
    "fp32r_dense": build_nc_fp32r_dense,
    "fp16_q": build_nc_fp16_q,
    "fp16_q2": lambda n_img=N_IMG: build_nc_fp16_q(
        n_img, po_bufs=2, psa4e_bufs=1, po_single=True),
    "fp16_q3": lambda n_img=N_IMG: build_nc_fp16_q(
        n_img, po_bufs=2, psa4e_bufs=1, po_single=True, drain_lag=1),
    "fp16_v2": build_nc_fp16_v2,
    "fp16_dmaonly": build_nc_dma_only,
}
_NC_CACHE = {}
DEFAULT_MODE = "fp16_q"


def round_f32r(a: np.ndarray) -> np.ndarray:
    """Round-to-nearest-even to fp32r's 11 explicit mantissa bits."""
    bits = np.ascontiguousarray(a, dtype=np.float32).view(np.uint32)
    r = bits + np.uint32(0x7FF) + ((bits >> np.uint32(12)) & np.uint32(1))
    r &= np.uint32(0xFFFFF000)
    return r.view(np.float32)


def get_nc(mode: str = "fp32_banded", n_img: int = N_IMG):
    key = (mode, n_img)
    if key not in _NC_CACHE:
        _NC_CACHE[key] = _BUILDERS[mode](n_img)
    return _NC_CACHE[key]


def prep_inputs(xs: np.ndarray, mode: str):
    """Mode-specific CPU-side dtype prep of (x-images, filter-matrix)."""
    mfull = build_filter_matrix()
    if mode.startswith("fp16"):
        return xs.astype(np.float16), mfull.astype(np.float16)
    if mode.startswith("fp32r"):
        return round_f32r(xs), mfull
    return xs, mfull


def kernel(x: np.ndarray) -> np.ndarray:
    assert x.shape == (4, 64, S, S), x.shape
    xs = np.ascontiguousarray(x, dtype=np.float32).reshape(256, S, S)
    xs, mfull = prep_inputs(xs, DEFAULT_MODE)
    nc = get_nc(DEFAULT_MODE, N_IMG)
    in_maps = [
        {"x": xs[c * N_IMG:(c + 1) * N_IMG], "m": mfull} for c in range(N_CORES)
    ]
    res = run_bass_kernel_spmd(nc, in_maps, list(range(N_CORES)))
    out = np.concatenate([res.results[c]["y"] for c in range(N_CORES)], axis=0)
    return out.astype(np.float32).reshape(4, 64, O, O)



# revision 35
# speedup vs baseline: 5.7920x; 5.7920x over previous
"""db2 DWT LL band (separable, symmetric pad, stride 2) on Trainium2.

Input  x: (4, 64, 512, 512) f32  ->  Output: (4, 64, 257, 257) f32.

Approach: the 1D low-pass+downsample along an axis is y = x @ M with a banded
matrix M [512, 257] (4 nonzeros per interior column; symmetric-pad edge columns
use combined coefficients). The separable 2D LL band is out = M^T @ img @ M.

On the PE (out = lhsT.T @ rhs, lhsT transposed for free):
  stage A: zT = matmul(lhsT=img, rhs=M)   -> zT[w, h'] (no transpose needed)
  stage B: out = matmul(lhsT=zT,  rhs=M)  -> out[h', w'] (final layout!)

Band sparsity: a 128-row tile of M only covers ~63 output columns, so each
(row-tile, col-range) pair is one matmul with N~64 instead of 257 - a 4x cut
in PE column-cycles, which makes *exact fp32* matmuls (4 cyc/row) affordable.
Row tiles overlap by 2 rows (stride 126) so every output column's 4-tap window
lives in exactly one tile; output column ranges are disjoint, PSUM handles the
scatter via start-once/overwrite semantics.

The leftover output row h'=256 (h' has 257 rows, PE M-dim chunks are 128+128+1)
is computed for all images at once on the DVE from the gathered last two input
rows (symmetric boundary => z = b*x[510] + a*x[511], then the W-filter).

Sharding: pure data parallel - 256 (b,c) images, 32 per core on 8 cores.
"""

import numpy as np
from contextlib import ExitStack

import bass_rust
import concourse.bass as bass
import concourse.bacc as bacc
import concourse.tile as tile
from concourse import mybir
from concourse.bass_utils import run_bass_kernel_spmd

F32 = mybir.dt.float32
F32R = mybir.dt.float32r
F16 = mybir.dt.float16

# db2 dec_lo
H0 = -0.12940952255092145
H1 = 0.22414386804185735
H2 = 0.8365163037378079
H3 = 0.48296291314469025
CA = H1 + H2  # symmetric-edge combined coefficients
CB = H0 + H3

S = 512     # input height/width
O = 257     # output height/width per axis
N_CORES = 8
N_IMG = 32  # images per core (256 total / 8 cores)

# overlapping row tiles (stride 126) and the disjoint output-column range each covers
ROW_TILES = [(0, 128), (126, 254), (252, 380), (378, 506), (504, 512)]
COL_RANGES = [(0, 64), (64, 127), (127, 190), (190, 253), (253, 257)]

MULT = mybir.AluOpType.mult
ADD = mybir.AluOpType.add


def build_filter_matrix() -> np.ndarray:
    m = np.zeros((S, O), dtype=np.float32)
    m[0, 0], m[1, 0] = CA, CB
    for i in range(1, 256):
        m[2 * i - 2, i] = H3
        m[2 * i - 1, i] = H2
        m[2 * i, i] = H1
        m[2 * i + 1, i] = H0
    m[510, 256], m[511, 256] = CB, CA
    return m


def _emit_row256(nc, tc, ctx, x, y, n_img):
    """out[:, 256, :] for all images: z = CB*x[510] + CA*x[511], then W-filter."""
    rp = ctx.enter_context(tc.tile_pool(name="r256", bufs=1))
    r510 = rp.tile([n_img, S], F32, tag="r510")
    nc.sync.dma_start(r510[:], x[:, 510, :].bitcast(F32))
    r511 = rp.tile([n_img, S], F32, tag="r511")
    nc.sync.dma_start(r511[:], x[:, 511, :].bitcast(F32))
    t0 = rp.tile([n_img, S], F32, tag="t0")
    nc.vector.tensor_scalar_mul(t0[:], r510[:], CB)
    z = rp.tile([n_img, S], F32, tag="z")
    nc.vector.scalar_tensor_tensor(z[:], r511[:], CA, t0[:], op0=MULT, op1=ADD)

    a0 = rp.tile([n_img, 255], F32, tag="a0")
    a1 = rp.tile([n_img, 255], F32, tag="a1")
    y256 = rp.tile([n_img, O], F32, tag="y256")
    # interior columns 1..255: y[i] = H3*z[2i-2] + H2*z[2i-1] + H1*z[2i] + H0*z[2i+1]
    nc.vector.tensor_scalar_mul(a0[:], z[:, 0:510:2], H3)
    nc.vector.scalar_tensor_tensor(a1[:], z[:, 1:511:2], H2, a0[:], op0=MULT, op1=ADD)
    nc.vector.scalar_tensor_tensor(a0[:], z[:, 2:512:2], H1, a1[:], op0=MULT, op1=ADD)
    nc.vector.scalar_tensor_tensor(y256[:, 1:256], z[:, 3:512:2], H0, a0[:], op0=MULT, op1=ADD)
    # edge columns
    c0 = rp.tile([n_img, 1], F32, tag="c0")
    nc.vector.tensor_scalar_mul(c0[:], z[:, 0:1], CA)
    nc.vector.scalar_tensor_tensor(y256[:, 0:1], z[:, 1:2], CB, c0[:], op0=MULT, op1=ADD)
    c1 = rp.tile([n_img, 1], F32, tag="c1")
    nc.vector.tensor_scalar_mul(c1[:], z[:, 510:511], CB)
    nc.vector.scalar_tensor_tensor(y256[:, 256:257], z[:, 511:512], CA, c1[:], op0=MULT, op1=ADD)
    nc.sync.dma_start(y[:, 256, :], y256[:])


def build_nc_fp32_banded(n_img: int = N_IMG):
    nc = bacc.Bacc("TRN2", target_bir_lowering=False, debug=False, num_devices=N_CORES)
    x = nc.declare_dram_parameter("x", [n_img, S, S], F32, isOutput=False)
    m = nc.declare_dram_parameter("m", [S, O], F32, isOutput=False)
    y = nc.declare_dram_parameter("y", [n_img, O, O], F32, isOutput=True)

    with tile.TileContext(nc) as tc:
        with ExitStack() as ctx:
            mp = ctx.enter_context(tc.tile_pool(name="mp", bufs=1))
            xp = ctx.enter_context(tc.tile_pool(name="xp", bufs=3))
            zp = ctx.enter_context(tc.tile_pool(name="zp", bufs=2))
            op = ctx.enter_context(tc.tile_pool(name="op", bufs=2))
            psa = ctx.enter_context(tc.tile_pool(name="psa", bufs=1, space="PSUM"))
            psb = ctx.enter_context(tc.tile_pool(name="psb", bufs=1, space="PSUM"))

            # banded filter blocks: mb[j] = M[rows RT[j], cols CR[j]]
            mb = []
            for j, (r0, r1) in enumerate(ROW_TILES):
                c0, c1 = COL_RANGES[j]
                t = mp.tile([r1 - r0, c1 - c0], F32, tag=f"m{j}")
                nc.sync.dma_start(t[:], m[r0:r1, c0:c1])
                mb.append(t)

            prev = None  # (zt tiles, image idx) pending stage B
            for n in range(n_img + 1):
                pz = None
                if n < n_img:
                    xt = []
                    for j, (r0, r1) in enumerate(ROW_TILES):
                        t = xp.tile([r1 - r0, S], F32, tag=f"x{j}")
                        nc.sync.dma_start(t[:], x[n, r0:r1, :])
                        xt.append(t)
                    # stage A: zT tile per w-slice; disjoint col writes per row-tile
                    pz = []
                    for i, (w0, w1) in enumerate(ROW_TILES):
                        p = psa.tile([w1 - w0, 256], F32, tag=f"z{i}")
                        for j, (r0, r1) in enumerate(ROW_TILES):
                            c0, c1 = COL_RANGES[j]
                            c1a = min(c1, 256)
                            nc.tensor.matmul(
                                p[:, c0:c1a], xt[j][:, w0:w1], mb[j][:, 0:c1a - c0],
                                start=(j == 0), stop=(j == len(ROW_TILES) - 1))
                        pz.append(p)
                po = None
                if prev is not None:
                    ztp, _ = prev
                    po = []
                    for mbi in range(2):
                        p = psb.tile([128, O], F32, tag=f"o{mbi}")
                        for i, (w0, w1) in enumerate(ROW_TILES):
                            c0, c1 = COL_RANGES[i]
                            nc.tensor.matmul(
                                p[:, c0:c1], ztp[i][:, mbi * 128:(mbi + 1) * 128], mb[i][:],
                                start=(i == 0), stop=(i == len(ROW_TILES) - 1))
                        po.append(p)
                if n < n_img:
                    zt = []
                    for i, (w0, w1) in enumerate(ROW_TILES):
                        t = zp.tile([w1 - w0, 256], F32, tag=f"zt{i}")
                        eng = nc.vector.tensor_copy if i % 2 == 0 else nc.scalar.copy
                        eng(t[:], pz[i][:])
                        zt.append(t)
                if prev is not None:
                    _, pn = prev
                    for mbi in range(2):
                        t = op.tile([128, O], F32, tag=f"ot{mbi}")
                        eng = nc.scalar.copy if mbi == 0 else nc.vector.tensor_copy
                        eng(t[:], po[mbi][:])
                        nc.sync.dma_start(y[pn, mbi * 128:(mbi + 1) * 128, :], t[:])
                prev = (zt, n) if n < n_img else None

            _emit_row256(nc, tc, ctx, x, y, n_img)
    nc.compile()
    return nc


def build_nc_fp32r_dense(n_img: int = N_IMG):
    nc = bacc.Bacc("TRN2", target_bir_lowering=False, debug=False, num_devices=N_CORES)
    x = nc.declare_dram_parameter("x", [n_img, S, S], F32R, isOutput=False)
    m = nc.declare_dram_parameter("m", [S, O], F32, isOutput=False)
    y = nc.declare_dram_parameter("y", [n_img, O, O], F32, isOutput=True)

    with tile.TileContext(nc) as tc:
        with ExitStack() as ctx:
            mp = ctx.enter_context(tc.tile_pool(name="mp", bufs=1))
            xp = ctx.enter_context(tc.tile_pool(name="xp", bufs=3))
            zp = ctx.enter_context(tc.tile_pool(name="zp", bufs=2))
            op = ctx.enter_context(tc.tile_pool(name="op", bufs=2))
            psa = ctx.enter_context(tc.tile_pool(name="psa", bufs=1, space="PSUM"))
            psb = ctx.enter_context(tc.tile_pool(name="psb", bufs=1, space="PSUM"))

            # M chunks as fp32r (round once via DVE). 258 cols: fp32r moving
            # free-dim must be even; col 257 is zero padding.
            mr = []
            for j in range(4):
                tf = mp.tile([128, O], F32, tag=f"mf{j}")
                nc.sync.dma_start(tf[:], m[j * 128:(j + 1) * 128, :])
                tr = mp.tile([128, 258], F32, tag=f"mr{j}")
                nc.vector.memset(tr[:, 256:258], 0.0)
                nc.vector.tensor_copy(tr[:, 0:257].bitcast(F32R), tf[:])
                mr.append(tr)

            prev = None
            for n in range(n_img + 1):
                pz = None
                if n < n_img:
                    xr = []
                    for j in range(4):
                        t = xp.tile([128, S], F32R, tag=f"x{j}")
                        nc.sync.dma_start(t[:], x[n, j * 128:(j + 1) * 128, :])
                        xr.append(t)
                    pz = []
                    for i in range(4):
                        p = psa.tile([128, 256], F32, tag=f"z{i}")
                        for j in range(4):
                            nc.tensor.matmul(
                                p[:], xr[j][:, i * 128:(i + 1) * 128],
                                mr[j][:, 0:256].bitcast(F32R),
                                start=(j == 0), stop=(j == 3))
                        pz.append(p)
                po = None
                if prev is not None:
                    ztp, _ = prev
                    po = []
                    for mbi in range(2):
                        p = psb.tile([128, 258], F32, tag=f"o{mbi}")
                        for i in range(4):
                            nc.tensor.matmul(
                                p[:], ztp[i][:, mbi * 128:(mbi + 1) * 128].bitcast(F32R),
                                mr[i][:].bitcast(F32R),
                                start=(i == 0), stop=(i == 3))
                        po.append(p)
                if n < n_img:
                    zt = []
                    for i in range(4):
                        t = zp.tile([128, 256], F32, tag=f"zt{i}")
                        eng = nc.vector.tensor_copy if i % 2 == 0 else nc.scalar.copy
                        eng(t[:].bitcast(F32R), pz[i][:])
                        zt.append(t)
                if prev is not None:
                    _, pn = prev
                    for mbi in range(2):
                        t = op.tile([128, O], F32, tag=f"ot{mbi}")
                        eng = nc.scalar.copy if mbi == 0 else nc.vector.tensor_copy
                        eng(t[:], po[mbi][:, 0:257])
                        nc.sync.dma_start(y[pn, mbi * 128:(mbi + 1) * 128, :], t[:])
                prev = (zt, n) if n < n_img else None

            _emit_row256(nc, tc, ctx, x, y, n_img)
    nc.compile()
    return nc



def build_nc_fp16_q(n_img: int = N_IMG, po_bufs: int = 1, psa4e_bufs: int = 2,
                    po_single: bool = False, drain_lag: int = 0,
                    split_engines: bool = False, repeats: int = 1,
                    zt_bufs: int = 3, xp_bufs: int = 3):
    """fp16 end-to-end, banded stride-126 tiling, quad-grouped output DMAs.

    Input x is pre-rounded to fp16 on CPU (free); output y is fp16 in HBM and
    upcast on CPU. All matmuls fp16 (1 cyc/row on PE) with f32 PSUM accum.

    Layouts (per quad of Q=4 images):
      xtS [128, Q*4*512]: xtS[p, (k*4+b)*512+w] = x[n0+k, 126b+p, w]
          (overlapping stride-126 h-blocks b=0..3, raw-AP DMA per image)
      xt4 [8, Q*512]:     xt4[p, k*512+w]       = x[n0+k, 504+p, w]
      mt5 [128, 5*257]:   mt5[p, b*257+c]       = M[126b+p, c]  (b=4: p<8)

    PSUM (8 banks): psA0 x2 = zT w-slices 0,1 | psA1 x2 = slices 2,3 |
      psA4E x2 = slice 4 ([0:8, 0:256]) + zT h'=256 edge col per slice
      ([:, 256+s]) | po = out h' rows, both 128-halves | perE = out row 256
      (per quad, [0:Q, 0:257]) + out w'=256 edge col per image ([:, 257+2k+hb]).
    All matmuls keep base partition 0 (HW requires base in {0,32,64}); psum
    group start/stop land on full-128-partition matmuls so the zero-region
    flags open and clear consistently.
    """
    assert n_img % 4 == 0
    Q = 4
    nc = bacc.Bacc("TRN2", target_bir_lowering=False, debug=False, num_devices=N_CORES)
    x = nc.declare_dram_parameter("x", [n_img, S, S], F16, isOutput=False)
    m = nc.declare_dram_parameter("m", [S, O], F16, isOutput=False)
    y = nc.declare_dram_parameter("y", [n_img, O, O], F16, isOutput=True)

    def emit_group(mms):
        for q, (o_, l_, r_) in enumerate(mms):
            nc.tensor.matmul(o_, l_, r_, start=(q == 0), stop=(q == len(mms) - 1))

    with tile.TileContext(nc) as tc:
        with ExitStack() as ctx:
            mp = ctx.enter_context(tc.tile_pool(name="mp", bufs=1))
            xp = ctx.enter_context(tc.tile_pool(name="xp", bufs=xp_bufs))
            zp = ctx.enter_context(tc.tile_pool(name="zp", bufs=2))
            op = ctx.enter_context(tc.tile_pool(name="op", bufs=2))
            ps = ctx.enter_context(tc.tile_pool(name="ps", bufs=1, space="PSUM"))

            def K_(b):
                return ROW_TILES[b][1] - ROW_TILES[b][0]

            mt5 = mp.tile([128, 5 * O], F16, tag="mt5")
            for b in range(5):
                r0, r1 = ROW_TILES[b]
                nc.sync.dma_start(mt5[0:r1 - r0, b * O:(b + 1) * O], m[r0:r1, :])

            def mmain(b, cap=None):
                c0, c1 = COL_RANGES[b]
                if cap is not None:
                    c1 = min(c1, cap)
                return mt5[0:K_(b), b * O + c0: b * O + c1]

            medge = mt5[0:8, 4 * O + 256: 4 * O + 257]

            n_quads = n_img // Q
            quads = {}    # g -> dict(xtS, xt4, zte, opt, perE, poE_last)
            pending = {}  # image t -> dict(ztA, ztB, zt4, g, k)
            pending_drain = {}  # image t -> (po, opt, k): deferred po drain
            # (repeats>1: timing-only builds re-run the whole pipeline)

            _rep_i = [0]

            def issue_quad_dmas(g):
                n0 = g * Q
                rg = f"{_rep_i[0]}_{g}"
                xtS = xp.tile([128, Q * 4 * S], F16, tag="xtS", name=f"xtS{rg}")
                for k in range(Q):
                    src = x[n0 + k].copy()
                    src.ap = bass_rust.VecI64Pair(
                        [[S, 128], [126 * S, 4], [1, S]])
                    nc.sync.dma_start(
                        xtS[:, k * 4 * S:(k + 1) * 4 * S].rearrange(
                            "p (b w) -> p b w", b=4), src)
                xt4 = xp.tile([8, Q * S], F16, tag="xt4", name=f"xt4{rg}")
                nc.sync.dma_start(
                    xt4[:].rearrange("p (n w) -> p n w", n=Q),
                    x[n0:n0 + Q, 504:512, :].rearrange("n p w -> p n w"))
                quads[g] = {"xtS": xtS, "xt4": xt4}

            def stage_a(t):
                g, k = t // Q, t % Q
                qd = quads[g]
                xtS, xt4 = qd["xtS"], qd["xt4"]
                if k == 0:
                    qd["zte"] = zp.tile([128, Q * 5], F16, tag="zte",
                                        name=f"zte{_rep_i[0]}_{g}")
                zte = qd["zte"]

                def xb(b, w0, w1):
                    if b == 4:
                        return xt4[0:8, k * S + w0: k * S + w1]
                    o0 = (k * 4 + b) * S
                    return xtS[:, o0 + w0: o0 + w1]

                pzA = ps.tile([128, 512], F32, tag="psA0", bufs=2)
                pzB = ps.tile([128, 512], F32, tag="psA1", bufs=2)
                pz4E = ps.tile([128, 512], F32, tag="psA4E", bufs=psa4e_bufs)
                gA, gB, g4 = [], [], []
                edges = []
                for s in range(5):
                    w0, w1 = ROW_TILES[s]
                    P = w1 - w0
                    dst, cb, grp = [(pzA, 0, gA), (pzA, 256, gA),
                                    (pzB, 0, gB), (pzB, 256, gB),
                                    (pz4E, 0, g4)][s]
                    for b in range(5):
                        c0, c1 = COL_RANGES[b]
                        c1a = min(c1, 256)
                        grp.append((dst[0:P, cb + c0: cb + c1a],
                                    xb(b, w0, w1), mmain(b, cap=256)))
                    edges.append((pz4E[0:P, 256 + s: 257 + s],
                                  xb(4, w0, w1), medge))
                emit_group(gA)
                emit_group(gB)
                # psA4E bank group: P=128 edge first (start) and last (stop);
                # the P=8 slice-4 work sits mid-group
                emit_group([edges[0]] + g4 + [edges[4], edges[1],
                                              edges[2], edges[3]])

                # zT copies stay off Pool so the stage-B output copies
                # (Pool) can't back-pressure the A->B chain via in-order
                # engine streams
                ztA = zp.tile([128, 512], F16, tag="ztA", bufs=zt_bufs)
                nc.vector.tensor_copy(ztA[:], pzA[:])
                ztB = zp.tile([128, 512], F16, tag="ztB", bufs=zt_bufs)
                if split_engines:
                    # DVE owns the big A-copies; Act owns small A-copies +
                    # all B-drains: A-chain never queues behind B-work
                    nc.vector.tensor_copy(ztB[:], pzB[:])
                else:
                    nc.scalar.copy(ztB[:], pzB[:])
                zt4 = zp.tile([8, 256], F16, tag="zt4", bufs=zt_bufs)
                eng4 = nc.scalar.copy if split_engines else nc.vector.tensor_copy
                eng4(zt4[:], pz4E[0:8, 0:256])
                nc.scalar.copy(zte[:, k * 5:k * 5 + 4], pz4E[:, 256:260])
                nc.scalar.copy(zte[0:8, k * 5 + 4:k * 5 + 5],
                               pz4E[0:8, 260:261])
                pending[t] = {"ztA": ztA, "ztB": ztB, "zt4": zt4}

            def stage_b(t):
                g, k = t // Q, t % Q
                qd = quads[g]
                pd = pending.pop(t)
                ztA, ztB, zt4 = pd["ztA"], pd["ztB"], pd["zt4"]
                if k == 0:
                    qd["opt"] = op.tile([128, Q * 2 * O], F16, tag="opt",
                                        name=f"opt{_rep_i[0]}_{g}", bufs=3)
                    qd["perE"] = ps.tile([128, 512], F32, tag="perE",
                                         name=f"perE{_rep_i[0]}_{g}")
                opt, perE = qd["opt"], qd["perE"]

                def zb(s, h0, h1):
                    if s < 2:
                        return ztA[:, s * 256 + h0: s * 256 + h1]
                    if s < 4:
                        return ztB[:, (s - 2) * 256 + h0: (s - 2) * 256 + h1]
                    return zt4[0:8, h0:h1]

                po = ps.tile([128, 512], F32, tag="po", bufs=po_bufs)
                gb = []
                for hb in range(2):
                    for s in range(5):
                        c0, c1 = COL_RANGES[s]
                        c1a = min(c1, 256)
                        gb.append((po[:, hb * 256 + c0: hb * 256 + c1a],
                                   zb(s, hb * 128, hb * 128 + 128),
                                   mmain(s, cap=256)))
                emit_group(gb)
                # out w'=256 edge col -> perE[:, 257+2k+hb]; the perE bank
                # group opens at image 0 and closes after the per-quad
                # edge-row matmuls (deferred last poE carries the stop)
                for hb in range(2):
                    mm = (perE[:, 257 + 2 * k + hb: 258 + 2 * k + hb],
                          zb(4, hb * 128, hb * 128 + 128), medge)
                    if k == Q - 1 and hb == 1:
                        qd["poE_last"] = mm
                    else:
                        nc.tensor.matmul(*mm, start=(k == 0 and hb == 0),
                                         stop=False)

                # Pool/GPSIMD can't read PSUM -> DVE/Act do the po drain
                if drain_lag:
                    pending_drain[t] = (po, opt, k)
                elif po_single:
                    # one instruction, alternate engine per image
                    eng = nc.vector.tensor_copy if t % 2 == 0 else nc.scalar.copy
                    eng(opt[:, k * 2 * O:(k + 1) * 2 * O].rearrange(
                            "p (b w) -> p b w", b=2)[:, :, 0:256],
                        po[:].rearrange("p (b w) -> p b w", b=2))
                else:
                    nc.vector.tensor_copy(
                        opt[:, k * 2 * O: k * 2 * O + 256], po[:, 0:256])
                    nc.scalar.copy(
                        opt[:, k * 2 * O + O: k * 2 * O + O + 256], po[:, 256:512])

            def quad_tail(g):
                qd = quads[g]
                zte, opt, perE = qd["zte"], qd["opt"], qd["perE"]
                # out row 256 (h'=256) for the quad, from zte
                for s in range(5):
                    P = K_(s)
                    c0, c1 = COL_RANGES[s]
                    nc.tensor.matmul(perE[0:Q, c0:c1], zte[0:P, s:Q * 5:5],
                                     mmain(s), start=False, stop=False)
                nc.tensor.matmul(*qd["poE_last"], start=False, stop=True)

                ert = op.tile([Q, O], F16, tag="ert", bufs=3)
                nc.scalar.copy(ert[:], perE[0:Q, 0:257])
                qd["ert"] = ert
                nc.scalar.copy(
                    opt[:].rearrange("p (n b w) -> p n b w", n=Q, b=2)[:, :, :, 256],
                    perE[:, 257:257 + 2 * Q].rearrange("p (n b) -> p n b", n=Q))

            def quad_out_dmas(g):
                # one step after quad_tail: by now every opt/ert write has
                # landed, so these never block SP's in-order queue (keeping
                # input prefetch flowing)
                n0 = g * Q
                qd = quads[g]
                opt, ert = qd["opt"], qd["ert"]
                nc.sync.dma_start(y[n0:n0 + Q, 256, :], ert[:])
                for hb in range(2):
                    nc.sync.dma_start(
                        y[n0:n0 + Q, hb * 128:(hb + 1) * 128, :].rearrange(
                            "n p w -> p n w"),
                        opt[:].rearrange("p (n b w) -> p n b w",
                                         n=Q, b=2)[:, :, hb, :])
                del quads[g]

            # software pipeline with lag 2: A(t) and A(t+1) run before
            # B(t), so the zT psum->sbuf copies of image t complete well
            # before stage B consumes them and the PE never stalls;
            # input DMAs prefetch one full quad ahead
            def drain_po(t):
                po, opt_, k_ = pending_drain.pop(t)
                if split_engines:
                    eng = nc.scalar.copy
                else:
                    eng = nc.vector.tensor_copy if t % 2 == 0 else nc.scalar.copy
                eng(opt_[:, k_ * 2 * O:(k_ + 1) * 2 * O].rearrange(
                        "p (b w) -> p b w", b=2)[:, :, 0:256],
                    po[:].rearrange("p (b w) -> p b w", b=2))

            for _rep in range(repeats):
                _rep_i[0] = _rep
                issue_quad_dmas(0)
                if n_quads > 1:
                    issue_quad_dmas(1)
                for t in range(n_img + 6):
                    if drain_lag and (t - 2 - drain_lag) in pending_drain:
                        drain_po(t - 2 - drain_lag)
                    if t < n_img:
                        if t % Q == 0 and t // Q + 2 < n_quads:
                            issue_quad_dmas(t // Q + 2)
                        stage_a(t)
                    if 2 <= t < n_img + 2:
                        stage_b(t - 2)
                        if (t - 2) % Q == Q - 1:
                            quad_tail((t - 2) // Q)
                    if t >= 4 and (t - 4) % Q == Q - 1:
                        quad_out_dmas((t - 4) // Q)
    nc.compile()
    return nc


def build_nc_fp16_v2(n_img: int = N_IMG):
    """Non-overlapping 128-row tiling variant of fp16_q.

    Both filter axes are tiled in 4 aligned 128-chunks (no stride-126
    overlap): band-boundary output columns (64/128/192) get contributions
    from two adjacent chunks, accumulated in PSUM via has_written adds.
    Kills the 8-row tail path entirely: the h'=256 / w'=256 edge columns
    (M rows 510/511) live in chunk 3, so the only extra matmuls are the
    per-slice zT-edge column (K=128, N=1) into a small pzE bank.

    Per image: 1 input DMA (fully contiguous 512KB), 20+10 matmuls,
    4 psum->sbuf copies. PSUM: pzA x2, pzB x2, pzE x1, po x2, perE x1 = 8.
    The po drain is deferred one pipeline step so it never convoys the
    stage-A copies on the in-order DVE/Act queues.
    """
    assert n_img % 4 == 0
    Q = 4
    HB = [(0, 128), (128, 256), (256, 384), (384, 512)]   # chunks (both axes)
    CR = [(0, 65), (64, 129), (128, 193), (192, 257)]     # cols per chunk
    nc = bacc.Bacc("TRN2", target_bir_lowering=False, debug=False, num_devices=N_CORES)
    x = nc.declare_dram_parameter("x", [n_img, S, S], F16, isOutput=False)
    m = nc.declare_dram_parameter("m", [S, O], F16, isOutput=False)
    y = nc.declare_dram_parameter("y", [n_img, O, O], F16, isOutput=True)

    def emit_group(mms):
        for q, (o_, l_, r_) in enumerate(mms):
            nc.tensor.matmul(o_, l_, r_, start=(q == 0), stop=(q == len(mms) - 1))

    with tile.TileContext(nc) as tc:
        with ExitStack() as ctx:
            mp = ctx.enter_context(tc.tile_pool(name="mp", bufs=1))
            xp = ctx.enter_context(tc.tile_pool(name="xp", bufs=3))
            zp = ctx.enter_context(tc.tile_pool(name="zp", bufs=2))
            op = ctx.enter_context(tc.tile_pool(name="op", bufs=2))
            ps = ctx.enter_context(tc.tile_pool(name="ps", bufs=1, space="PSUM"))

            mt4 = mp.tile([128, 4 * O], F16, tag="mt4")
            for b in range(4):
                nc.sync.dma_start(mt4[:, b * O:(b + 1) * O],
                                  m[b * 128:(b + 1) * 128, :])

            def mmain(b, cap=None):
                c0, c1 = CR[b]
                if cap is not None:
                    c1 = min(c1, cap)
                return mt4[:, b * O + c0: b * O + c1]

            medge = mt4[:, 3 * O + 256: 3 * O + 257]  # M[384:512, 256]

            n_quads = n_img // Q
            quads = {}
            pending = {}
            pending_drain = {}

            def issue_quad_dmas(g):
                n0 = g * Q
                xtS = xp.tile([128, Q * 4 * S], F16, tag="xtS", name=f"xtS{g}")
                for k in range(Q):
                    src = x[n0 + k].copy()
                    src.ap = bass_rust.VecI64Pair(
                        [[S, 128], [128 * S, 4], [1, S]])
                    nc.sync.dma_start(
                        xtS[:, k * 4 * S:(k + 1) * 4 * S].rearrange(
                            "p (b w) -> p b w", b=4), src)
                quads[g] = {"xtS": xtS}

            def stage_a(t):
                g, k = t // Q, t % Q
                qd = quads[g]
                xtS = qd["xtS"]
                if k == 0:
                    qd["zte"] = zp.tile([128, Q * 4], F16, tag="zte",
                                        name=f"zte{g}")
                zte = qd["zte"]

                def xb(b, w0, w1):
                    o0 = (k * 4 + b) * S
                    return xtS[:, o0 + w0: o0 + w1]

                pzA = ps.tile([128, 512], F32, tag="psA0", bufs=2)
                pzB = ps.tile([128, 512], F32, tag="psA1", bufs=2)
                pzE = ps.tile([128, 4], F32, tag="psE", bufs=1)
                gA, gB, gE = [], [], []
                for s in range(4):
                    w0, w1 = HB[s]
                    dst, cb, grp = [(pzA, 0, gA), (pzA, 256, gA),
                                    (pzB, 0, gB), (pzB, 256, gB)][s]
                    for b in range(4):
                        c0, c1 = CR[b]
                        c1a = min(c1, 256)
                        grp.append((dst[:, cb + c0: cb + c1a],
                                    xb(b, w0, w1), mmain(b, cap=256)))
                    gE.append((pzE[:, s:s + 1], xb(3, w0, w1), medge))
                emit_group(gA)
                emit_group(gB)
                emit_group(gE)

                ztA = zp.tile([128, 512], F16, tag="ztA", bufs=3)
                nc.vector.tensor_copy(ztA[:], pzA[:])
                ztB = zp.tile([128, 512], F16, tag="ztB", bufs=3)
                nc.scalar.copy(ztB[:], pzB[:])
                nc.scalar.copy(zte[:, k * 4:(k + 1) * 4], pzE[:])
                pending[t] = {"ztA": ztA, "ztB": ztB}

            def stage_b(t):
                g, k = t // Q, t % Q
                qd = quads[g]
                pd = pending.pop(t)
                ztA, ztB = pd["ztA"], pd["ztB"]
                if k == 0:
                    qd["opt"] = op.tile([128, Q * 2 * O], F16, tag="opt",
                                        name=f"opt{g}", bufs=3)
                    qd["perE"] = ps.tile([128, 512], F32, tag="perE",
                                         name=f"perE{g}")
                opt, perE = qd["opt"], qd["perE"]

                def zb(s, h0, h1):
                    if s < 2:
                        return ztA[:, s * 256 + h0: s * 256 + h1]
                    return ztB[:, (s - 2) * 256 + h0: (s - 2) * 256 + h1]

                po = ps.tile([128, 512], F32, tag="po", bufs=2)
                gb = []
                for hb in range(2):
                    for s in range(4):
                        c0, c1 = CR[s]
                        c1a = min(c1, 256)
                        gb.append((po[:, hb * 256 + c0: hb * 256 + c1a],
                                   zb(s, hb * 128, hb * 128 + 128),
                                   mmain(s, cap=256)))
                emit_group(gb)
                # out w'=256 edge col -> perE[:, 257+2k+hb]
                for hb in range(2):
                    mm = (perE[:, 257 + 2 * k + hb: 258 + 2 * k + hb],
                          zb(3, hb * 128, hb * 128 + 128), medge)
                    if k == Q - 1 and hb == 1:
                        qd["poE_last"] = mm
                    else:
                        nc.tensor.matmul(*mm, start=(k == 0 and hb == 0),
                                         stop=False)
                pending_drain[t] = (po, opt, k)

            def drain_po(t):
                po, opt_, k_ = pending_drain.pop(t)
                eng = nc.vector.tensor_copy if t % 2 == 0 else nc.scalar.copy
                eng(opt_[:, k_ * 2 * O:(k_ + 1) * 2 * O].rearrange(
                        "p (b w) -> p b w", b=2)[:, :, 0:256],
                    po[:].rearrange("p (b w) -> p b w", b=2))

            def quad_tail(g):
                qd = quads[g]
                zte, opt, perE = qd["zte"], qd["opt"], qd["perE"]
                # out row h'=256 for the quad (incl. the w'=256 corner)
                for s in range(4):
                    c0, c1 = CR[s]
                    nc.tensor.matmul(perE[0:Q, c0:c1], zte[:, s:Q * 4:4],
                                     mmain(s), start=False, stop=False)
                nc.tensor.matmul(*qd["poE_last"], start=False, stop=True)

                ert = op.tile([Q, O], F16, tag="ert", bufs=3)
                nc.scalar.copy(ert[:], perE[0:Q, 0:257])
                qd["ert"] = ert
                nc.scalar.copy(
                    opt[:].rearrange("p (n b w) -> p n b w", n=Q, b=2)[:, :, :, 256],
                    perE[:, 257:257 + 2 * Q].rearrange("p (n b) -> p n b", n=Q))

            def quad_out_dmas(g):
                n0 = g * Q
                qd = quads[g]
                opt, ert = qd["opt"], qd["ert"]
                nc.sync.dma_start(y[n0:n0 + Q, 256, :], ert[:])
                for hb in range(2):
                    nc.sync.dma_start(
                        y[n0:n0 + Q, hb * 128:(hb + 1) * 128, :].rearrange(
                            "n p w -> p n w"),
                        opt[:].rearrange("p (n b w) -> p n b w",
                                         n=Q, b=2)[:, :, hb, :])
                del quads[g]

            issue_quad_dmas(0)
            if n_quads > 1:
                issue_quad_dmas(1)
            for t in range(n_img + 7):
                if (t - 3) in pending_drain:
                    drain_po(t - 3)
                if t < n_img:
                    if t % Q == 0 and t // Q + 2 < n_quads:
                        issue_quad_dmas(t // Q + 2)
                    stage_a(t)
                if 2 <= t < n_img + 2:
                    stage_b(t - 2)
                    if (t - 2) % Q == Q - 1:
                        quad_tail((t - 2) // Q)
                if t >= 5 and (t - 5) % Q == Q - 1:
                    quad_out_dmas((t - 5) // Q)
    nc.compile()
    return nc


def build_nc_dma_only(n_img: int = N_IMG, repeats: int = 1):
    """DMA floor probe: same input/output DMA traffic as fp16_q, no compute.

    Output data is garbage (a zero tile DMA'd out) - only for run_time.
    """
    assert n_img % 4 == 0
    Q = 4
    nc = bacc.Bacc("TRN2", target_bir_lowering=False, debug=False, num_devices=N_CORES)
    x = nc.declare_dram_parameter("x", [n_img, S, S], F16, isOutput=False)
    m = nc.declare_dram_parameter("m", [S, O], F16, isOutput=False)
    y = nc.declare_dram_parameter("y", [n_img, O, O], F16, isOutput=True)

    with tile.TileContext(nc) as tc:
        with ExitStack() as ctx:
            mp = ctx.enter_context(tc.tile_pool(name="mp", bufs=1))
            xp = ctx.enter_context(tc.tile_pool(name="xp", bufs=3))
            op = ctx.enter_context(tc.tile_pool(name="op", bufs=1))
            mt = mp.tile([128, O], F16, tag="mt")
            nc.sync.dma_start(mt[:], m[0:128, :])
            opt = op.tile([128, Q * 2 * O], F16, tag="opt")
            nc.vector.memset(opt[:], 0.0)
            ert = op.tile([Q, O], F16, tag="ert")
            nc.vector.memset(ert[:], 0.0)
            for _rep in range(repeats):
                for g in range(n_img // Q):
                    n0 = g * Q
                    xtS = xp.tile([128, Q * 4 * S], F16, tag="xtS",
                                  name=f"xtS{_rep}_{g}")
                    for k in range(Q):
                        src = x[n0 + k].copy()
                        src.ap = bass_rust.VecI64Pair(
                            [[S, 128], [128 * S, 4], [1, S]])
                        nc.sync.dma_start(
                            xtS[:, k * 4 * S:(k + 1) * 4 * S].rearrange(
                                "p (b w) -> p b w", b=4), src)
                    nc.sync.dma_start(y[n0:n0 + Q, 256, :], ert[:])
                    for hb in range(2):
                        nc.sync.dma_start(
                            y[n0:n0 + Q, hb * 128:(hb + 1) * 128, :].rearrange(
                                "n p w -> p n w"),
                            opt[:].rearrange("p (n b w) -> p n b w",
                                             n=Q, b=2)[:, :, hb, :])
    nc.compile()
    return nc


_BUILDERS = {
    "fp32_banded": build_nc_fp32_banded,
    "fp32r_dense": build_nc_fp32r_dense,
    "fp16_q": build_nc_fp16_q,
    "fp16_q2": lambda n_img=N_IMG, repeats=1: build_nc_fp16_q(
        n_img, po_bufs=2, psa4e_bufs=1, po_single=True, repeats=repeats),
    "fp16_q3": lambda n_img=N_IMG, repeats=1: build_nc_fp16_q(
        n_img, po_bufs=2, psa4e_bufs=1, po_single=True, drain_lag=1,
        repeats=repeats),
    "fp16_q6": lambda n_img=N_IMG, repeats=1: build_nc_fp16_q(
        n_img, po_bufs=2, psa4e_bufs=1, po_single=True, drain_lag=1,
        split_engines=True, repeats=repeats),
    "fp16_q7": lambda n_img=N_IMG, repeats=1: build_nc_fp16_q(
        n_img, po_bufs=2, psa4e_bufs=1, po_single=True, drain_lag=1,
        repeats=repeats, zt_bufs=4, xp_bufs=4),
    "fp16_v2": build_nc_fp16_v2,
    "fp16_dmaonly": build_nc_dma_only,
}
_NC_CACHE = {}
DEFAULT_MODE = "fp16_q3"


def round_f32r(a: np.ndarray) -> np.ndarray:
    """Round-to-nearest-even to fp32r's 11 explicit mantissa bits."""
    bits = np.ascontiguousarray(a, dtype=np.float32).view(np.uint32)
    r = bits + np.uint32(0x7FF) + ((bits >> np.uint32(12)) & np.uint32(1))
    r &= np.uint32(0xFFFFF000)
    return r.view(np.float32)


def get_nc(mode: str = "fp32_banded", n_img: int = N_IMG):
    key = (mode, n_img)
    if key not in _NC_CACHE:
        base, reps = mode, 1
        if ".r" in mode:
            base, r = mode.rsplit(".r", 1)
            reps = int(r)
        if reps == 1:
            _NC_CACHE[key] = _BUILDERS[base](n_img)
        else:
            _NC_CACHE[key] = _BUILDERS[base](n_img, repeats=reps)
    return _NC_CACHE[key]


def prep_inputs(xs: np.ndarray, mode: str):
    """Mode-specific CPU-side dtype prep of (x-images, filter-matrix)."""
    mfull = build_filter_matrix()
    if mode.startswith("fp16"):
        return xs.astype(np.float16), mfull.astype(np.float16)
    if mode.startswith("fp32r"):
        return round_f32r(xs), mfull
    return xs, mfull


def kernel(x: np.ndarray) -> np.ndarray:
    assert x.shape == (4, 64, S, S), x.shape
    xs = np.ascontiguousarray(x, dtype=np.float32).reshape(256, S, S)
    xs, mfull = prep_inputs(xs, DEFAULT_MODE)
    nc = get_nc(DEFAULT_MODE, N_IMG)
    in_maps = [
        {"x": xs[c * N_IMG:(c + 1) * N_IMG], "m": mfull} for c in range(N_CORES)
    ]
    res = run_bass_kernel_spmd(nc, in_maps, list(range(N_CORES)))
    out = np.concatenate([res.results[c]["y"] for c in range(N_CORES)], axis=0)
    return out.astype(np.float32).reshape(4, 64, O, O)

